# revision 50
# baseline (speedup 1.0000x reference)
"""Causal self-attention for Trainium2, 8 NeuronCores.

Sharding: tensor-parallel over heads (4 heads/core) x data-parallel over
batch (2). Core i handles batch i//4, heads 4*(i%4)..4*(i%4)+3. Each core
computes its heads' attention output and a partial output projection
(W_proj rows for its heads); the host sums the 4 partials per batch and
adds b_proj.

Device layout choices:
  - Q^T, K^T computed feature-major [dim, t] directly (lhsT = W chunk,
    rhs = x^T chunk), so attention scores come out as S^T [k, q] with k
    on partitions -- which is exactly the layout the P@V matmul needs
    as its rhs. No on-chip transposes of the O(T^2) object.
  - V computed in natural [t, dim] layout (lhsT = x^T chunk, rhs = W_v),
    which is the lhsT layout the P@V matmul needs. A ones-column is
    appended to V so the softmax denominators fall out of the same
    matmul (row 64*... of the PSUM output).
  - exp() without max subtraction: scores are q.k/8 with q,k ~ N(0,1),
    bounded well inside fp32 exp range; softmax is shift-invariant so
    the result is mathematically identical to the reference.
  - all matmuls run as float32r (replicated fp32) with free dim >= 256,
    which streams at 1 column/cycle like bf16.

The causal mask is handled by skipping fully-masked k-chunks and
multiplying exp(S) by one of 4 precomputed 0/1 indicator tiles on the
diagonal-straddling chunks. If the runtime mask is not the lower-tri
causal mask, a general fallback multiplies by the actual mask (DMA'd
transposed) instead; an all-ones mask drops masking entirely.
"""

import numpy as np

B, T, C, H = 2, 2048, 1024, 16
D = C // H            # 64 head dim
NCORES = 8
NBG = 2               # batch shards
NHG = 4               # head-group shards
HL = H // NHG         # 4 heads per core
DL = HL * D           # 256 local feature dims
NDQ = DL // 128       # 2 partition chunks of local dims
NTB = T // 512        # 4 t-chunks of 512
NKC = T // 128        # 16 key chunks of 128
NQC = T // 512        # 4 query chunks of 512
NTT = T // 128        # 16 t-tiles of 128 (proj / V)

_CACHE = {}


def _build(mode, debug_dump=False):
    """Build + compile the per-core Bass program. mode: causal|full|general."""
    import concourse.bass as bass
    import concourse.bacc as bacc
    import concourse.tile as tile
    import concourse.mybir as mybir

    f32 = mybir.dt.float32
    bf16 = mybir.dt.bfloat16
    Exp = mybir.ActivationFunctionType.Exp
    Ident = mybir.ActivationFunctionType.Identity
    mult = mybir.AluOpType.mult
    add = mybir.AluOpType.add

    nc = bacc.Bacc(
        "TRN2", target_bir_lowering=False, debug=False, num_devices=NCORES
    )

    xT = nc.dram_tensor("xT", [C, T], bf16, kind="ExternalInput").ap()
    Wl = nc.dram_tensor("Wl", [C, 3 * DL], bf16, kind="ExternalInput").ap()
    bqk = nc.dram_tensor("bqk", [128, 2 * NDQ], f32, kind="ExternalInput").ap()
    bv = nc.dram_tensor("bv", [1, DL], f32, kind="ExternalInput").ap()
    Wp = nc.dram_tensor("Wp", [DL, C], bf16, kind="ExternalInput").ap()
    maskT = None
    if mode == "general":
        maskT = nc.dram_tensor("maskT", [T, T], bf16, kind="ExternalInput").ap()
    yp = nc.dram_tensor("yp", [T, C], bf16, kind="ExternalOutput").ap()
    dbg = {}
    if debug_dump:
        for nm, shp, dt in [
            ("qt_d", [128, NDQ, T], bf16), ("kt_d", [128, NDQ, T], bf16),
            ("v1_d", [128, NKC, HL, D + 1], bf16), ("ot_d", [128, NDQ, T], bf16),
            ("st_d", [128, 512], f32), ("p_d", [128, 512], bf16),
            ("o_d", [65, 512], f32),
        ]:
            dbg[nm] = nc.dram_tensor(nm, shp, dt, kind="ExternalOutput").ap()

    with tile.TileContext(nc) as tc:
        with (
            tc.tile_pool(name="singles", bufs=1) as singles,
            tc.tile_pool(name="xin", bufs=2) as xin,
            tc.tile_pool(name="ptiles", bufs=6) as ptiles,
            tc.tile_pool(name="small", bufs=4) as small,
            tc.tile_pool(name="outp", bufs=3) as outp,
            tc.tile_pool(name="psum", bufs=7, space="PSUM") as psum,
        ):
            def ps512(name):
                return psum.tile(
                    [128, 512], f32, name="ps512", tag="ps512", bufs=4
                )

            # ---- resident inputs ----
            # W and x loads split per kc-chunk so the first matmuls can
            # start as soon as their chunk lands. Each dma_start costs
            # ~600ns of issue time on its queue, so W goes on sync while
            # x goes on gpsimd (parallel issue streams).
            W_sb = singles.tile([128, 8, 3 * DL], bf16)
            Wl_r = Wl.rearrange("(kc p) n -> p kc n", p=128)
            x0_sb = xin.tile([128, 8, 512], bf16, tag="x_sb", name="x_sb")
            x0r = xT.rearrange("(kc p) t -> p kc t", p=128)[:, :, 0:512]
            # tiny fast-path load of kc=0's Q columns so the very first
            # matmul (s=0, dq=0 chain) starts ~2us earlier
            nc.sync.dma_start(out=W_sb[:, 0, 0:128], in_=Wl_r[:, 0, 0:128])
            for kc in range(8):
                src = Wl_r[:, kc, :] if kc > 0 else Wl_r[:, 0, 128:]
                dst = W_sb[:, kc, :] if kc > 0 else W_sb[:, 0, 128:]
                nc.sync.dma_start(out=dst, in_=src)
                # gpsimd is free earliest (scalar first loads the exp
                # table), so it takes the even chunks including kc=0
                eng = nc.gpsimd if kc % 2 == 0 else nc.scalar
                eng.dma_start(out=x0_sb[:, kc, :], in_=x0r[:, kc, :])
            bqk_sb = singles.tile([128, 2 * NDQ], f32)
            nc.scalar.dma_start(out=bqk_sb, in_=bqk)
            bv_row = singles.tile([1, DL], f32)
            nc.scalar.dma_start(out=bv_row, in_=bv)
            bv_sb = singles.tile([128, DL], f32)
            nc.gpsimd.partition_broadcast(bv_sb, bv_row)

            ind = None
            if mode == "causal":
                ind = singles.tile([128, 4, 512], bf16)
                for j in range(4):
                    nc.vector.memset(ind[:, j, :], 1.0)
                    # keep (=1.0) iff f - p - 128*j >= 0, else 0.0
                    nc.gpsimd.affine_select(
                        out=ind[:, j, :],
                        in_=ind[:, j, :],
                        compare_op=mybir.AluOpType.is_ge,
                        fill=0.0,
                        base=-128 * j,
                        pattern=[[1, 512]],
                        channel_multiplier=-1,
                    )

            # ---- resident intermediates ----
            QT = singles.tile([128, NDQ, T], bf16)   # [dim%128, dimchunk, t]
            KT = singles.tile([128, NDQ, T], bf16)
            V1 = singles.tile([128, NKC, HL, D + 1], bf16)  # [t%128, kc, h, d+1]
            nc.vector.memset(V1[:, :, :, D : D + 1], 1.0)
            OT = singles.tile([128, NDQ, T], bf16)
            # bf16 denominator rows (partition 64) + a ones row there for
            # the PE-based partition broadcast in norm_hp
            stageb = singles.tile([65, NQC, HL, 512], bf16)
            stagef = singles.tile([65, NQC, HL, 512], f32)
            ones64b = singles.tile([65, 64], bf16)
            nc.vector.memset(ones64b[64:65, :], 1.0)
            Wp_sb = singles.tile([128, NDQ, C], bf16)

            # ---- phase 1: QKV projections (as interleavable units) ----
            def p1_units(tb, x_sb):
                """Units for one 512-wide t-chunk of the QKV projection."""
                units = []
                if tb > 0:
                    def dma_u(tb=tb, x_sb=x_sb):
                        # prefetched well ahead of use: 2 coarse DMAs keep
                        # queue issue time low (vs 8 fine-grained ones)
                        xr = xT.rearrange("(kc p) t -> p kc t", p=128)[
                            :, :, tb * 512 : (tb + 1) * 512
                        ]
                        for g in range(2):
                            nc.gpsimd.dma_start(
                                out=x_sb[:, 4 * g : 4 * g + 4, :],
                                in_=xr[:, 4 * g : 4 * g + 4, :],
                            )
                    units.append(dma_u)
                for s in range(2):  # 0=Q, 1=K
                    for dq in range(NDQ):
                        def qk_u(tb=tb, s=s, dq=dq, x_sb=x_sb):
                            ps = ps512("qk")
                            col = s * DL + dq * 128
                            for kc in range(8):
                                nc.tensor.matmul(
                                    ps,
                                    lhsT=W_sb[:, kc, col : col + 128],
                                    rhs=x_sb[:, kc, :],
                                    start=(kc == 0),
                                    stop=(kc == 7),
                                )
                            dst = (QT if s == 0 else KT)[
                                :, dq, tb * 512 : (tb + 1) * 512
                            ]
                            nc.vector.tensor_scalar_add(
                                dst, ps,
                                bqk_sb[:, s * NDQ + dq : s * NDQ + dq + 1],
                            )
                        units.append(qk_u)
                for t4 in range(4):
                    def v_u(tb=tb, t4=t4, x_sb=x_sb):
                        tt = tb * 4 + t4
                        ps = ps512("v")
                        for kc in range(8):
                            nc.tensor.matmul(
                                ps[:, :DL],
                                lhsT=x_sb[:, kc, t4 * 128 : (t4 + 1) * 128],
                                rhs=W_sb[:, kc, 2 * DL : 3 * DL],
                                start=(kc == 0),
                                stop=(kc == 7),
                            )
                        nc.vector.tensor_tensor(
                            out=V1[:, tt, :, 0:D],
                            in0=ps[:, :DL].rearrange("p (h d) -> p h d", d=D),
                            in1=bv_sb.rearrange("p (h d) -> p h d", d=D),
                            op=add,
                        )
                    units.append(v_u)
                return units

            # ---- phase 2/3 units ----
            def proj_units(qc, tail=False):
                # one unit per (row-tile, output-half): ~0.85us of PE work
                # each, small enough to fit the exp-wait bubbles of the
                # last qc's attention without displacing it
                units = []
                for t4 in range(4):
                    st = {}
                    def u(qc=qc, t4=t4, st=st, n=0):
                        tt = qc * 4 + t4
                        if n == 0:
                            st["y"] = outp.tile([128, C], bf16, name="y_sb")
                        y_sb = st["y"]
                        pp = ps512("proj")
                        for dq in range(NDQ):
                            nc.tensor.matmul(
                                pp,
                                lhsT=OT[:, dq, tt * 128 : (tt + 1) * 128],
                                rhs=Wp_sb[:, dq, n * 512 : (n + 1) * 512],
                                start=(dq == 0),
                                stop=(dq == NDQ - 1),
                            )
                        # in the tail batch scalar is exp-free, so use it
                        # as a second PSUM drain; back-to-back proj units
                        # are otherwise paced by the copies
                        if tail and n == 1:
                            nc.scalar.activation(
                                y_sb[:, n * 512 : (n + 1) * 512], pp, Ident
                            )
                        else:
                            nc.vector.tensor_copy(
                                y_sb[:, n * 512 : (n + 1) * 512], pp
                            )
                        if n == 1:
                            nc.sync.dma_start(
                                out=yp[tt * 128 : (tt + 1) * 128, :],
                                in_=y_sb,
                            )
                    units.append(lambda u=u: u(n=0))
                    units.append(lambda u=u: u(n=1))
                return units

            def attn_units(qc, hp, nkc, m_sb):
                """One head-pair's attention over all k-chunks, softmax
                denominators via the ones-column of V1."""
                state = {}

                def emit_mm1(j):
                    # diagonal-straddling chunks: columns < 128*jd are fully
                    # masked, so stream only the live q range.
                    qo = 0
                    if mode == "causal" and j >= 4 * qc:
                        qo = 128 * (j - 4 * qc)
                    stp = psum.tile(
                        [128, 2, 512], f32, name="ps1024", tag="ps1024", bufs=2
                    )
                    for hh in range(2):
                        off = 64 * hh
                        nc.tensor.matmul(
                            stp[:, hh, qo:],
                            lhsT=KT[off : off + 64, hp, j * 128 : (j + 1) * 128],
                            rhs=QT[
                                off : off + 64, hp, qc * 512 + qo : (qc + 1) * 512
                            ],
                            start=True,
                            stop=True,
                        )
                    state.setdefault("st", {})[j] = stp

                def prologue():
                    state["ops"] = [ps512("o"), ps512("o")]
                    state["emitted"] = min(2, nkc)  # lookahead 1
                    for j in range(state["emitted"]):
                        emit_mm1(j)

                def consume(kc):
                    ops = state["ops"]
                    stp = state["st"].pop(kc)
                    p2 = ptiles.tile([128, 2, 512], bf16, tag="p")
                    if mode == "causal" and kc >= 4 * qc:
                        # exp only the columns the causal mask can reach,
                        # then apply the diagonal indicator to both heads
                        # at once. The masked prefix is never computed or
                        # consumed.
                        j = kc - 4 * qc
                        qo = 128 * j
                        nc.scalar.activation(
                            p2[:, :, qo:], stp[:, :, qo:], Exp
                        )
                        base = ind[:, j, qo:]
                        ind2 = bass.AP(
                            tensor=base.tensor,
                            offset=base.offset,
                            ap=[base.ap[0], [0, 2], base.ap[1]],
                        )
                        nc.vector.tensor_tensor(
                            out=p2[:, :, qo:],
                            in0=p2[:, :, qo:],
                            in1=ind2,
                            op=mult,
                        )
                        for hh in range(2):
                            h = hp * 2 + hh
                            # start=True lazily zeroes the whole 2KB PSUM
                            # zero region (per partition), so only the first
                            # chunk carries it; unwritten bytes stay pending
                            # and later chunks' writes overwrite them. The
                            # flags act region-wide, so partial-width
                            # instructions carry them fine.
                            nc.tensor.matmul(
                                ops[hh][: D + 1, qo:],
                                lhsT=V1[:, kc, h, :],
                                rhs=p2[:, hh, qo:],
                                start=(kc == 0),
                                stop=(kc == nkc - 1),
                            )
                    else:
                        nc.scalar.activation(p2, stp, Exp)
                        if mode == "general":
                            base = m_sb[:, kc, :]
                            msk2 = bass.AP(
                                tensor=base.tensor,
                                offset=base.offset,
                                ap=[base.ap[0], [0, 2], base.ap[1]],
                            )
                            nc.vector.tensor_tensor(
                                out=p2, in0=p2, in1=msk2, op=mult
                            )
                        for hh in range(2):
                            h = hp * 2 + hh
                            nc.tensor.matmul(
                                ops[hh][: D + 1, :],
                                lhsT=V1[:, kc, h, :],
                                rhs=p2[:, hh, :],
                                start=(kc == 0),
                                stop=(kc == nkc - 1),
                            )
                    if state["emitted"] < nkc:
                        emit_mm1(state["emitted"])
                        state["emitted"] += 1

                def stash():
                    # unnormalized output rows + denominators (the
                    # denominator row stays on partition 64 -- DVE
                    # can't move data across partitions)
                    ops = state["ops"]
                    # denominator rows first: they're tiny and they're all
                    # norm_hp needs to start its broadcast, which then
                    # overlaps the long OT copies. (Custom DVE ops like
                    # reciprocal_approx_* silently produce garbage on HW
                    # when the AP base partition isn't 0, so the
                    # reciprocal happens downstream at partition 0.)
                    for hh in range(2):
                        h = hp * 2 + hh
                        dst = stageb if qc == NQC - 1 else stagef
                        nc.vector.tensor_copy(
                            dst[64:65, qc, h, :], ops[hh][D : D + 1, :]
                        )
                    for hh in range(2):
                        off = 64 * hh
                        nc.vector.tensor_copy(
                            OT[off : off + 64, hp, qc * 512 : (qc + 1) * 512],
                            ops[hh][0:D, :],
                        )

                units = [prologue]
                for kc in range(nkc):
                    units.append(lambda kc=kc: consume(kc))
                units.append(stash)
                return units

            # DRAM scratch for the mid-kernel denominator broadcasts (SBUF
            # sources can't have partition-step 0; DRAM sources can)
            rcp_dram = nc.dram_tensor(
                "rcp_scratch", [NQC, NDQ, NDQ, 512], bf16, kind="Internal"
            ).ap()

            def norm_hp(qc, hp):
                # per-(qc, head-pair) normalization; runs as soon as this
                # pair's reciprocals are stashed. Engines can't move data
                # across partitions, so the partition-64 reciprocal row is
                # broadcast either by a DRAM bounce (no PE cost, ~5us
                # latency -- fine mid-kernel) or by the PE itself (K=1
                # outer product with a ones column, ~0.4us of PE time but
                # ~1us total latency) for the latency-critical last qc.
                if qc == NQC - 1:
                    # PE-broadcast the raw denominators, then reciprocal
                    # at base partition 0 (where custom DVE ops work)
                    psB = ps512("rb")
                    for hh in range(2):
                        nc.tensor.matmul(
                            psB[64 * hh : 64 * hh + 64, :],
                            lhsT=ones64b[64:65, 0:64],
                            rhs=stageb[64:65, qc, 2 * hp + hh, :],
                            start=True,
                            stop=True,
                        )
                    rcp = small.tile([128, 512], f32, tag="rcp2", bufs=2)
                    nc.vector.reciprocal_approx_fast(out=rcp, in_=psB)
                    nc.vector.tensor_tensor(
                        out=OT[:, hp, qc * 512 : (qc + 1) * 512],
                        in0=OT[:, hp, qc * 512 : (qc + 1) * 512],
                        in1=rcp,
                        op=mult,
                    )
                    return
                sums2 = small.tile([NDQ, 512], f32, tag="sums2", bufs=2)
                nc.gpsimd.dma_start(
                    out=sums2, in_=stagef[64:65, qc, 2 * hp : 2 * hp + 2, :]
                )
                rcp2 = small.tile([NDQ, 512], f32, tag="rcp2b", bufs=2)
                nc.vector.reciprocal_approx_fast(out=rcp2, in_=sums2)
                rcpb2 = small.tile([NDQ, 512], bf16, tag="rcpb2", bufs=2)
                nc.vector.tensor_copy(rcpb2, rcp2)
                nc.sync.dma_start(out=rcp_dram[qc, hp], in_=rcpb2)
                rb_hp = small.tile([128, 512], bf16, tag="rb", bufs=2)
                for hh in range(2):
                    src = rcp_dram[qc, hp, hh : hh + 1, :]
                    src = bass.AP(
                        tensor=src.tensor,
                        offset=src.offset,
                        ap=[[0, 64], src.ap[-1]],
                    )
                    eng = nc.gpsimd if hh == 0 else nc.sync
                    eng.dma_start(
                        out=rb_hp[64 * hh : 64 * hh + 64, :], in_=src
                    )
                nc.vector.tensor_tensor(
                    out=OT[:, hp, qc * 512 : (qc + 1) * 512],
                    in0=OT[:, hp, qc * 512 : (qc + 1) * 512],
                    in1=rb_hp,
                    op=mult,
                )

            # ---- schedule: staircase interleave ----
            # attn(qc) needs phase-1 chunks tb <= qc only, so phase-1(tb+1)
            # and proj(qc-1) units are injected between attention units to
            # keep the PE FIFO fed while ACT paces the exp chain.
            for u in p1_units(0, x0_sb):
                u()
            nc.sync.dma_start(
                out=Wp_sb, in_=Wp.rearrange("(dq p) n -> p dq n", p=128)
            )
            for qc in range(NQC):
                nkc = 4 * qc + 4 if mode == "causal" else NKC
                m_sb = None
                if mode == "general":
                    m_sb = xin.tile([128, NKC, 512], bf16, tag="mask", bufs=1)
                    nc.sync.dma_start(
                        out=m_sb,
                        in_=maskT.rearrange("(kc p) q -> p kc q", p=128)[
                            :, :, qc * 512 : (qc + 1) * 512
                        ],
                    )
                inj_early = []
                if qc + 1 < NTB:
                    x_next = xin.tile(
                        [128, 8, 512], bf16, tag="x_sb", name="x_sb"
                    )
                    inj_early += p1_units(qc + 1, x_next)
                inj_late = proj_units(qc - 1) if qc >= 1 else []
                main = []
                for hp in range(NDQ):
                    units = attn_units(qc, hp, nkc, m_sb)
                    units.append(lambda qc=qc, hp=hp: norm_hp(qc, hp))
                    main += units
                half = (len(main) + 1) // 2
                for part, inj in ((main[:half], inj_early), (main[half:], inj_late)):
                    k, m, j = len(part), len(inj), 0
                    for i, u in enumerate(part):
                        u()
                        take = (i + 1) * m // k - i * m // k
                        for _ in range(take):
                            inj[j]()
                            j += 1
            for u in proj_units(NQC - 1, tail=True):
                u()

            if debug_dump:
                nc.sync.dma_start(out=dbg["ot_d"], in_=OT)

    nc.compile()
    return nc


def _host_prep(x, prefix_causal_mask, W_attn, b_attn, W_proj):
    """Split full inputs into 8 per-core input maps; detect mask mode."""
    scale = 1.0 / np.sqrt(np.float32(D))
    mask = np.asarray(prefix_causal_mask)
    if mask.all():
        mode = "full"
    else:
        tri = np.tril(np.ones((T, T), dtype=bool))
        if all(np.array_equal(mask[b], tri) for b in range(B)):
            mode = "causal"
        else:
            mode = "general"

    import ml_dtypes

    bf16 = ml_dtypes.bfloat16
    x = np.asarray(x, dtype=np.float32)
    W_attn = np.asarray(W_attn, dtype=np.float32)
    b_attn = np.asarray(b_attn, dtype=np.float32)
    W_proj = np.asarray(W_proj, dtype=np.float32)

    in_maps = []
    for core in range(NCORES):
        b = core // NHG
        hg = core % NHG
        lo = hg * DL
        hi = lo + DL
        xT = np.ascontiguousarray(x[b].T)  # [C, T]
        Wq = W_attn[:, lo:hi] * scale
        Wk = W_attn[:, C + lo : C + hi]
        Wv = W_attn[:, 2 * C + lo : 2 * C + hi]
        Wl = np.ascontiguousarray(np.concatenate([Wq, Wk, Wv], axis=1))
        bq = b_attn[lo:hi] * scale
        bk = b_attn[C + lo : C + hi]
        # bias per partition for Q,K chunks: cols = [q0, q1, k0, k1]
        bqk = np.stack(
            [bq[0:128], bq[128:256], bk[0:128], bk[128:256]], axis=1
        ).astype(np.float32)
        bv = np.ascontiguousarray(
            b_attn[2 * C + lo : 2 * C + hi][None, :]
        ).astype(np.float32)
        Wp = np.ascontiguousarray(W_proj[lo:hi, :])
        im = {
            "xT": xT.astype(bf16),
            "Wl": Wl.astype(bf16),
            "bqk": np.ascontiguousarray(bqk),
            "bv": bv,
            "Wp": Wp.astype(bf16),
        }
        if mode == "general":
            im["maskT"] = np.ascontiguousarray(mask[b].T).astype(bf16)
        in_maps.append(im)
    return mode, in_maps


def _get_program(mode):
    if mode not in _CACHE:
        _CACHE[mode] = _build(mode)
    return _CACHE[mode]


def _run(inputs, trace=False):
    """Returns (full_output [B,T,C], BassKernelResults)."""
    from concourse import bass_utils

    mode, in_maps = _host_prep(
        inputs["x"],
        inputs["prefix_causal_mask"],
        inputs["W_attn"],
        inputs["b_attn"],
        inputs["W_proj"],
    )
    nc = _get_program(mode)
    res = bass_utils.run_bass_kernel_spmd(
        nc, in_maps, core_ids=list(range(NCORES)), trace=trace
    )
    b_proj = np.asarray(inputs["b_proj"], dtype=np.float32)
    y = np.zeros((B, T, C), dtype=np.float32)
    for core in range(NCORES):
        y[core // NHG] += np.asarray(res.results[core]["yp"], dtype=np.float32)
    y += b_proj[None, None, :]
    return y, res


def kernel(**inputs):
    y, _ = _run(inputs, trace=False)
    return y

